# revision 1
# baseline (speedup 1.0000x reference)
"""Causal self-attention block (LN -> QKV -> causal attention -> out-proj)
on 8 Trainium2 NeuronCores.

Sharding: core = 2*batch + head_group. Each core handles one batch element
(S=2048 tokens) and 8 of the 16 heads (tensor-parallel split of w_qkv along
the head axis and w_out along its input dim). The two partial outputs per
batch are summed on the host (the all-reduce of the sharding hint).

Device kernel layout strategy (per core):
  - LayerNorm in natural layout [s, d], then PE-transpose to xnT [d, s]
    (contraction dim must sit on partitions for matmuls).
  - QKV projection computes q^T/k^T in [head_dim, s] layout directly and V in
    natural [s, head_dim] layout, so causal attention needs no further
    transposes: scores are computed transposed, ST[k, q] = k . q, softmax'd
    along the partition-free axis via exp + a ones-column appended to V
    (the PV matmul then yields both y^T and the softmax row-sums).
  - ln_scale/ln_bias/b_qkv/softmax-scale/b_out are all folded into the
    weights on the host; matmuls run as float32r (FP22, full PE rate).
"""

import os

# the device path runs through jax's axon PJRT plugin; make sure a
# pre-set JAX_PLATFORMS doesn't hide it (unset = all plugins load)
_jp = os.environ.get("JAX_PLATFORMS")
if _jp and "axon" not in _jp:
    os.environ["JAX_PLATFORMS"] = f"axon,{_jp}"

import numpy as np

import concourse.bass as bass
import concourse.mybir as mybir
import concourse.tile as tile
from concourse import bacc
from concourse.bass_utils import run_bass_kernel_spmd
from concourse.masks import make_identity

B, S, D, H, HD = 4, 2048, 1024, 16, 64
HL = H // 2          # heads per core (local)
NCH = D // 128       # 8 contraction chunks
NSB = S // 128       # 16 s-blocks
NQS = S // 512       # 4 q-superblocks
NEG = -1.0e38
LN_EPS = 1e-6

f32 = mybir.dt.float32
f32r = mybir.dt.float32r

_cache = {}


def build_program():
    nc = bacc.Bacc()

    x_d = nc.declare_dram_parameter("x", [S, D], f32, isOutput=False)
    wqk_d = nc.declare_dram_parameter("wqk", [NCH, 128, 1024], f32r, isOutput=False)
    wv_d = nc.declare_dram_parameter("wv", [NCH, 128, 512], f32r, isOutput=False)
    bqk_d = nc.declare_dram_parameter("bqk", [128, 2, 4], f32, isOutput=False)
    bv1_d = nc.declare_dram_parameter("bv1", [1, 512], f32r, isOutput=False)
    vones_d = nc.declare_dram_parameter("vones", [1, 128], f32r, isOutput=False)
    wout_d = nc.declare_dram_parameter("wout", [4, 128, 1024], f32r, isOutput=False)
    out_d = nc.declare_dram_parameter("out", [S, D], f32, isOutput=True)

    with tile.TileContext(nc, pool_alloc_mode="queue") as tc:
        with (
            tc.tile_pool(name="singles", bufs=1) as singles,
            tc.tile_pool(name="qkT", bufs=1) as qkTp,
            tc.tile_pool(name="vpool", bufs=1) as vpool,
            tc.tile_pool(name="pscm", bufs=1, space="PSUM") as pscm,
        ):
            # ---- constants ----
            ident = singles.tile([128, 128], f32)
            make_identity(nc, ident)
            identb = singles.tile([128, 128], mybir.dt.bfloat16)
            make_identity(nc, identb)
            maskTb = singles.tile([128, 128], mybir.dt.bfloat16)
            nc.gpsimd.memset(maskTb, 0.0)
            nc.gpsimd.affine_select(
                out=maskTb, in_=maskTb,
                compare_op=mybir.AluOpType.is_ge,
                fill=NEG, base=0,
                pattern=[[1, 128]], channel_multiplier=-1,
            )
            eps_t = singles.tile([128, 1], f32)
            nc.vector.memset(eps_t, LN_EPS)
            bqk_t = singles.tile([128, 2, 4], f32)
            nc.sync.dma_start(out=bqk_t, in_=bqk_d[:, :, :])
            bv1_t = singles.tile([1, 512], f32r)
            nc.sync.dma_start(out=bv1_t, in_=bv1_d[:, :])
            vones_t = singles.tile([1, 128], f32r)
            nc.sync.dma_start(out=vones_t, in_=vones_d[:, :])

            # ---- persistent activations ----
            qT = qkTp.tile([128, 4, S], f32r)   # [pair-row, pair, s]
            kT = qkTp.tile([128, 4, S], f32r)
            # V'' [s-row, s-block, head, 65] (col 64 = ones)
            vpp = vpool.tile([128, NSB, HL, HD + 1], f32r)
            nc.gpsimd.memset(vpp[:, :, :, HD : HD + 1].bitcast(f32), 1.0)

            # ================= Phase A: LayerNorm + transpose =================
            with tc.tile_pool(name="xnT", bufs=1) as xnTp:
                xnT = xnTp.tile([128, NCH, S], f32r)
                with (
                    tc.tile_pool(name="atmp", bufs=5) as atmp,
                    tc.tile_pool(name="astat", bufs=8) as astat,
                ):
                    for i in range(NSB):
                        x_t = atmp.tile([128, D], f32, tag="x")
                        nc.sync.dma_start(out=x_t, in_=x_d[i * 128 : (i + 1) * 128, :])
                        stats = astat.tile([128, 2, 6], f32, tag="stats")
                        nc.vector.bn_stats(out=stats[:, 0, :], in_=x_t[:, 0:512])
                        nc.vector.bn_stats(out=stats[:, 1, :], in_=x_t[:, 512:1024])
                        mv = astat.tile([128, 2], f32, tag="mv")
                        nc.vector.bn_aggr(out=mv, in_=stats)
                        std_t = astat.tile([128, 1], f32, tag="std")
                        nc.scalar.activation(
                            out=std_t, in_=mv[:, 1:2],
                            func=mybir.ActivationFunctionType.Sqrt,
                            bias=eps_t, scale=1.0,
                        )
                        rstd_t = astat.tile([128, 1], f32, tag="rstd")
                        nc.vector.reciprocal(out=rstd_t, in_=std_t)
                        xn_t = atmp.tile([128, D], f32, tag="xn")
                        nc.vector.tensor_scalar(
                            out=xn_t, in0=x_t,
                            scalar1=mv[:, 0:1], scalar2=rstd_t,
                            op0=mybir.AluOpType.subtract, op1=mybir.AluOpType.mult,
                        )
                        for c4 in range(0, NCH, 4):
                            pst = pscm.tile([128, 4, 128], f32, tag="yt", bufs=4)
                            for c in range(c4, c4 + 4):
                                nc.tensor.transpose(
                                    pst[:, c - c4, :],
                                    xn_t[:, c * 128 : (c + 1) * 128],
                                    ident,
                                )
                            nc.scalar.activation(
                                out=xnT[:, c4 : c4 + 4, i * 128 : (i + 1) * 128],
                                in_=pst,
                                func=mybir.ActivationFunctionType.Copy,
                            )

                # ================= Phase B: QKV projection =================
                with (
                    tc.tile_pool(name="wqk", bufs=2) as wqkp,
                    tc.tile_pool(name="wvp", bufs=1) as wvp,
                ):
                    def emit_qk(t, p):
                        fb = t * 4 + p
                        w_t = wqkp.tile([128, NCH, 128], f32r, tag="wqk",
                                        name=f"wqk_{t}_{p}")
                        nc.sync.dma_start(
                            out=w_t,
                            in_=wqk_d[:, :, fb * 128 : (fb + 1) * 128].rearrange(
                                "c d f -> d c f"
                            ),
                        )
                        dest = qT if t == 0 else kT
                        for sb in range(NQS):
                            ps = pscm.tile([128, 512], f32, tag="st", bufs=2,
                                           name=f"psqk_{t}_{p}_{sb}")
                            for c in range(NCH):
                                nc.tensor.matmul(
                                    ps,
                                    w_t[:, c, :],
                                    xnT[:, c, sb * 512 : (sb + 1) * 512],
                                    start=(c == 0),
                                    stop=(c == NCH - 1),
                                )
                            nc.vector.tensor_scalar_add(
                                out=dest[:, p, sb * 512 : (sb + 1) * 512],
                                in0=ps,
                                scalar1=bqk_t[:, t, p : p + 1],
                            )

                    def emit_v():
                        wv_t = wvp.tile([128, NCH, 512], f32r)
                        for c in range(NCH):
                            nc.sync.dma_start(out=wv_t[:, c, :], in_=wv_d[c, :, :])
                        for i in range(NSB):
                            psv = pscm.tile([128, 512], f32, tag="st", bufs=2,
                                            name=f"psv_{i}")
                            for c in range(NCH):
                                nc.tensor.matmul(
                                    psv,
                                    xnT[:, c, i * 128 : (i + 1) * 128],
                                    wv_t[:, c, :],
                                    start=(c == 0),
                                    stop=False,
                                )
                            # += ones[s] x bv  (rank-1 bias update)
                            nc.tensor.matmul(
                                psv, vones_t, bv1_t, start=False, stop=True,
                            )
                            nc.vector.tensor_copy(
                                vpp[:, i, :, 0:HD],
                                psv.rearrange("p (h v) -> p h v", v=HD),
                            )

                    # pair 0 first, then V, so attention on heads 0/1 can
                    # start while the rest of the projection still runs
                    emit_qk(0, 0)
                    emit_qk(1, 0)
                    emit_v()
                    for p in range(1, 4):
                        emit_qk(0, p)
                        emit_qk(1, p)

            # ================= Phase C: causal attention =================
            with tc.tile_pool(name="ytall", bufs=1) as ytallp:
                ytall = ytallp.tile([128, 4, S], f32r)  # [pair-row, pair, s]
                with (
                    tc.tile_pool(name="ptp", bufs=6) as ptp,
                    tc.tile_pool(name="ctmp", bufs=4) as ctmp,
                    tc.tile_pool(name="dscr", bufs=8, space="DRAM") as dscr,
                    tc.tile_pool(name="woutp", bufs=1) as woutp,
                    tc.tile_pool(name="ypool", bufs=3) as ypool,
                ):
                    wout_t = woutp.tile([128, 4, 1024], f32r)
                    for c in range(4):
                        nc.sync.dma_start(out=wout_t[:, c, :], in_=wout_d[c, :, :])
                    def emit_outproj(i):
                        y_t = ypool.tile([128, 1024], f32, tag="y",
                                         name=f"y_{i}")
                        for nh in range(2):
                            # alternate tags: the yt slots are idle during
                            # the output projection, use them for depth
                            pso = pscm.tile([128, 512], f32,
                                            tag=("st" if nh == 0 else "yt"),
                                            bufs=(2 if nh == 0 else 4),
                                            name=f"pso_{i}_{nh}")
                            for c in range(4):
                                nc.tensor.matmul(
                                    pso,
                                    ytall[:, c, i * 128 : (i + 1) * 128],
                                    wout_t[:, c, nh * 512 : (nh + 1) * 512],
                                    start=(c == 0),
                                    stop=(c == 3),
                                )
                            nc.vector.tensor_copy(
                                y_t[:, nh * 512 : (nh + 1) * 512], pso
                            )
                        nc.sync.dma_start(
                            out=out_d[i * 128 : (i + 1) * 128, :], in_=y_t
                        )

                    for sb in range(NQS):
                        for p in range(4):
                            # the pair's two heads (PE rows 0:64 / 64:128)
                            # run as adjacent matmuls -> concurrent row-groups
                            q0 = sb * 512
                            jmax = 4 * sb + 3
                            yts = [
                                pscm.tile([HD + 1, 512], f32, tag="yt",
                                          bufs=4, name=f"yt_{2 * p + hf}_{sb}")
                                for hf in range(2)
                            ]
                            for j in range(jmax + 1):
                                r = max(0, j - 4 * sb)
                                diag = j >= 4 * sb
                                L = 512 - 128 * r
                                st = pscm.tile([128, 1024], f32, tag="st",
                                               bufs=2, name=f"st_{p}_{sb}_{j}")
                                pt = ptp.tile([128, 1024], f32r, tag="pt")
                                for hf in range(2):
                                    rows = slice(hf * HD, (hf + 1) * HD)
                                    # hf0 packs left in bank 0; hf1 must stay
                                    # bank-aligned at 512 (matmul outputs
                                    # cannot cross a PSUM bank boundary)
                                    lo = hf * 512
                                    nc.tensor.matmul(
                                        st[:, lo : lo + L],
                                        kT[rows, p, j * 128 : (j + 1) * 128],
                                        qT[rows, p, q0 + r * 128 : q0 + 512],
                                        start=True, stop=not diag,
                                    )
                                if diag:
                                    # causal mask folded in on the PE:
                                    # st[diag] += I.T @ maskT
                                    for hf in range(2):
                                        nc.tensor.matmul(
                                            st[:, hf * 512 : hf * 512 + 128],
                                            identb,
                                            maskTb,
                                            start=False, stop=True,
                                        )
                                # one wide exp across both heads (for r>0 the
                                # [L:512) strip is unread garbage)
                                nc.scalar.activation(
                                    out=pt[:, 0 : 512 + L],
                                    in_=st[:, 0 : 512 + L],
                                    func=mybir.ActivationFunctionType.Exp,
                                )
                                for hf in range(2):
                                    nc.tensor.matmul(
                                        yts[hf][:, r * 128 : 512],
                                        vpp[:, j, 2 * p + hf, :],
                                        pt[:, hf * 512 : hf * 512 + L],
                                        start=(j == 0),
                                        stop=(j == jmax),
                                    )
                            # per-superblock softmax normalization epilogue
                            for hf in range(2):
                                rows = slice(hf * HD, (hf + 1) * HD)
                                yt = yts[hf]
                                ssum = ctmp.tile([1, 512], f32, tag="ssum")
                                nc.vector.tensor_copy(ssum, yt[HD : HD + 1, :])
                                dsum = dscr.tile([512], f32, tag="dsum")
                                nc.sync.dma_start(out=dsum, in_=ssum)
                                sums4 = ctmp.tile([4, 128], f32, tag="sums4")
                                nc.sync.dma_start(
                                    out=sums4,
                                    in_=dsum.rearrange("(a b) -> a b", b=128),
                                )
                                sinv4 = ctmp.tile([4, 128], f32, tag="sinv4")
                                nc.vector.reciprocal(out=sinv4, in_=sums4)
                                dsinv = dscr.tile([512], f32, tag="dsinv")
                                nc.sync.dma_start(
                                    out=dsinv.rearrange("(a b) -> a b", b=128),
                                    in_=sinv4,
                                )
                                src = dsinv[:]
                                bcast = bass.AP(
                                    tensor=src.tensor,
                                    offset=src.offset,
                                    ap=[[0, HD]] + list(src.ap),
                                )
                                binv = ctmp.tile([HD, 512], f32, tag="binv")
                                nc.sync.dma_start(out=binv, in_=bcast)
                                nc.vector.tensor_mul(
                                    out=ytall[rows, p, q0 : q0 + 512],
                                    in0=yt[0:HD, :],
                                    in1=binv,
                                )

                    for i in range(NSB):
                        emit_outproj(i)

    nc.finalize()
    return nc


def _prep_core_inputs(x, ln_scale, ln_bias, w_qkv, b_qkv, w_out):
    """Host-side folding + per-core input maps."""
    scale = np.float32(HD ** -0.5)
    # qkv = xn@W + b_qkv, xn = z*ln_scale + ln_bias  =>  z @ (ln_scale*W) + (ln_bias@W + b_qkv)
    b_eff = b_qkv + np.einsum(
        "d,dhf->hf", ln_bias.astype(np.float64), w_qkv.astype(np.float64)
    ).astype(np.float32)
    w_eff = ln_scale[:, None, None] * w_qkv
    wq = w_eff[:, :, 0:64] * scale
    wk = w_eff[:, :, 64:128]
    wv = w_eff[:, :, 128:192]
    bq = b_eff[:, 0:64] * scale
    bk = b_eff[:, 64:128]
    bv = b_eff[:, 128:192]

    in_maps = []
    for core in range(8):
        b, g = core // 2, core % 2
        hsel = slice(g * HL, (g + 1) * HL)
        # [D, 4 pairs, 128] with head 2p in rows 0:64, head 2p+1 in 64:128
        qp = wq[:, hsel].reshape(D, 4, 128)
        kp = wk[:, hsel].reshape(D, 4, 128)
        wqk = np.concatenate(
            [qp.reshape(D, 512), kp.reshape(D, 512)], axis=1
        ).reshape(NCH, 128, 1024)
        wv_g = np.ascontiguousarray(wv[:, hsel].reshape(D, 512)).reshape(
            NCH, 128, 512
        )
        bq_p = bq[hsel].reshape(4, 128)
        bk_p = bk[hsel].reshape(4, 128)
        bqk = np.ascontiguousarray(
            np.stack([bq_p, bk_p], axis=0).transpose(2, 0, 1)
        )
        bv1 = np.ascontiguousarray(bv[hsel].reshape(1, 512))
        wout = np.ascontiguousarray(
            w_out[g * 512 : (g + 1) * 512, :].reshape(4, 128, 1024)
        )
        in_maps.append(
            {
                "x": np.ascontiguousarray(x[b]),
                "wqk": np.ascontiguousarray(wqk),
                "wv": wv_g,
                "bqk": bqk,
                "bv1": bv1,
                "vones": np.ones((1, 128), np.float32),
                "wout": wout,
            }
        )
    return in_maps


def kernel(x, mask, ln_scale, ln_bias, w_qkv, b_qkv, w_out, b_out, **run_kwargs):
    x = np.asarray(x, np.float32)
    ln_scale = np.asarray(ln_scale, np.float32)
    ln_bias = np.asarray(ln_bias, np.float32)
    w_qkv = np.asarray(w_qkv, np.float32)
    b_qkv = np.asarray(b_qkv, np.float32)
    w_out = np.asarray(w_out, np.float32)
    b_out = np.asarray(b_out, np.float32)
    if "nc" not in _cache:
        _cache["nc"] = build_program()
    nc = _cache["nc"]
    in_maps = _prep_core_inputs(x, ln_scale, ln_bias, w_qkv, b_qkv, w_out)
    res = run_bass_kernel_spmd(nc, in_maps, list(range(8)), **run_kwargs)
    _cache["last_result"] = res
    out = np.empty((B, S, D), np.float32)
    for b in range(B):
        out[b] = res.results[2 * b]["out"] + res.results[2 * b + 1]["out"]
    out += np.asarray(b_out)[None, None, :]
    return out



# revision 29
# speedup vs baseline: 1.1073x; 1.1073x over previous
"""Causal self-attention block (LN -> QKV -> causal attention -> out-proj)
on 8 Trainium2 NeuronCores.

Sharding: core = 2*batch + head_group. Each core handles one batch element
(S=2048 tokens) and 8 of the 16 heads (tensor-parallel split of w_qkv along
the head axis and w_out along its input dim). The two partial outputs per
batch are summed on the host (the all-reduce of the sharding hint).

Device kernel layout strategy (per core):
  - LayerNorm in natural layout [s, d]: bn_stats on DVE, the normalize is a
    ScalarE Identity activation with per-partition scale/bias, output bf16.
  - All transposes run on the DMA xbar (dma_start_transpose, 16x128 tiles):
    xn -> xnT [d, s] per 128-token block, and the attention outputs
    y_norm [q, f] -> ytall [f, q]. The PE never transposes.
  - QKV computes q^T/k^T in [head_dim, s] layout (bf16) and V in natural
    [s, head_dim] layout (bf16 + ones column for softmax row sums). The V
    projection is interleaved into the LayerNorm phase per block.
  - Scores computed transposed, ST[k, q] = k . q, causal mask folded in on
    the PE for diagonal blocks; exp on ScalarE -> probabilities pt (bf16).
  - PV runs with the probability block as the *stationary* operand and
    V' [128, 65] as the moving operand, accumulating y[q, hd] + row-sum in
    PSUM across k-blocks (65-wide matmuls). Softmax normalization is one
    broadcasted tensor_tensor against the reciprocal row sums.
  - The whole program is software-pipelined around the ScalarE exp (the
    second-busiest engine): PV lags scores by one k-block, and deferred PE
    work (QKV projection chunks, output-projection blocks) is drip-fed into
    the attention stream as filler via a cost-budgeted queue.
  - ln_scale/ln_bias/b_qkv/softmax-scale/b_out are folded into the weights
    on the host; weights shipped bf16.
"""

import os

# the device path runs through jax's axon PJRT plugin; make sure a
# pre-set JAX_PLATFORMS doesn't hide it (unset = all plugins load)
_jp = os.environ.get("JAX_PLATFORMS")
if _jp and "axon" not in _jp:
    os.environ["JAX_PLATFORMS"] = f"axon,{_jp}"

import ml_dtypes
import numpy as np

import concourse.bass as bass
import concourse.mybir as mybir
import concourse.tile as tile
from concourse import bacc
from concourse.bass_utils import run_bass_kernel_spmd
from concourse.masks import make_identity

B, S, D, H, HD = 4, 2048, 1024, 16, 64
HL = H // 2          # heads per core (local)
NCH = D // 128       # 8 contraction chunks
NSB = S // 128       # 16 s-blocks
NQS = S // 512       # 4 q-superblocks
NEG = -1.0e38
LN_EPS = 1e-6

f32 = mybir.dt.float32
f32r = mybir.dt.float32r
bf16 = mybir.dt.bfloat16
bfloat16 = ml_dtypes.bfloat16

_cache = {}


def build_program():
    nc = bacc.Bacc()

    x_d = nc.declare_dram_parameter("x", [S, D], f32, isOutput=False)
    wqk_d = nc.declare_dram_parameter("wqk", [NCH, 128, 1024], bf16, isOutput=False)
    wv_d = nc.declare_dram_parameter("wv", [NCH, 128, 512], bf16, isOutput=False)
    bqk_d = nc.declare_dram_parameter("bqk", [128, 2, 4], f32, isOutput=False)
    bv1_d = nc.declare_dram_parameter("bv1", [1, 512], f32r, isOutput=False)
    vones_d = nc.declare_dram_parameter("vones", [1, 128], f32r, isOutput=False)
    wout_d = nc.declare_dram_parameter("wout", [4, 128, 1024], bf16, isOutput=False)
    out_d = nc.declare_dram_parameter("out", [S, D], f32, isOutput=True)

    with tile.TileContext(nc, pool_alloc_mode="queue") as tc:
        with (
            tc.tile_pool(name="singles", bufs=1) as singles,
            tc.tile_pool(name="qkT", bufs=1) as qkTp,
            tc.tile_pool(name="vpool", bufs=1) as vpool,
            tc.tile_pool(name="xnT", bufs=1) as xnTp,
            tc.tile_pool(name="atmp", bufs=7) as atmp,
            tc.tile_pool(name="astat", bufs=8) as astat,
            tc.tile_pool(name="wqk", bufs=8) as wqkp,
            tc.tile_pool(name="wvp", bufs=1) as wvp,
            tc.tile_pool(name="ytall", bufs=1) as ytallp,
            tc.tile_pool(name="ptp", bufs=6) as ptp,
            tc.tile_pool(name="ctmp", bufs=4) as ctmp,
            tc.tile_pool(name="woutp", bufs=1) as woutp,
            tc.tile_pool(name="ypool", bufs=2) as ypool,
            tc.tile_pool(name="pscm", bufs=1, space="PSUM") as pscm,
        ):
            # ---- first input block DMA before anything else ----
            x_tiles = {}
            x_tiles[0] = atmp.tile([128, D], f32, tag="x", name="x_0")
            nc.sync.dma_start(out=x_tiles[0][:, 0:512], in_=x_d[0:128, 0:512])
            nc.sync.dma_start(out=x_tiles[0][:, 512:1024],
                              in_=x_d[0:128, 512:1024])

            wv_t0 = wvp.tile([128, NCH, 512], bf16)
            nc.sync.dma_start(out=wv_t0, in_=wv_d.rearrange("c d f -> d c f"))

            # ---- constants ----
            identb = singles.tile([128, 128], bf16)
            make_identity(nc, identb)
            maskTb = singles.tile([128, 128], bf16)
            nc.gpsimd.memset(maskTb, 0.0)
            nc.gpsimd.affine_select(
                out=maskTb, in_=maskTb,
                compare_op=mybir.AluOpType.is_ge,
                fill=NEG, base=0,
                pattern=[[1, 128]], channel_multiplier=-1,
            )
            eps_t = singles.tile([128, 1], f32)
            nc.vector.memset(eps_t, LN_EPS)
            bqk_t = singles.tile([128, 2, 4], f32)
            nc.sync.dma_start(out=bqk_t, in_=bqk_d[:, :, :])
            bv1_t = singles.tile([1, 512], f32r)
            nc.sync.dma_start(out=bv1_t, in_=bv1_d[:, :])
            vones_t = singles.tile([1, 128], f32r)
            nc.sync.dma_start(out=vones_t, in_=vones_d[:, :])

            # ---- persistent activations ----
            qT = qkTp.tile([128, 4, S], bf16)   # [pair-row, pair, s]
            kT = qkTp.tile([128, 4, S], bf16)
            # V'' [s-row, s-block, head, 65] (col 64 = ones)
            vpp = vpool.tile([128, NSB, HL, HD + 1], bf16)
            nc.gpsimd.memset(vpp[:, :, :, HD : HD + 1], 1.0)
            xnT = xnTp.tile([128, NCH, S], bf16)
            ytall = ytallp.tile([128, 4, S], bf16)  # [pair-row, pair, s]
            wv_t = wv_t0
            wout_t = woutp.tile([128, 4, 1024], bf16)

            # =========== per-block LayerNorm + transpose + V proj ===========
            def emit_A_block(i):
                if i in x_tiles:
                    x_t = x_tiles[i]
                else:
                    x_t = atmp.tile([128, D], f32, tag="x", name=f"x_{i}")
                    rows = slice(i * 128, (i + 1) * 128)
                    if i < 2:
                        nc.sync.dma_start(out=x_t[:, 0:512],
                                          in_=x_d[rows, 0:512])
                        nc.sync.dma_start(out=x_t[:, 512:1024],
                                          in_=x_d[rows, 512:1024])
                    else:
                        nc.sync.dma_start(out=x_t, in_=x_d[rows, :])
                stats = astat.tile([128, 2, 6], f32, tag="stats")
                nc.vector.bn_stats(out=stats[:, 0, :], in_=x_t[:, 0:512])
                nc.vector.bn_stats(out=stats[:, 1, :], in_=x_t[:, 512:1024])
                mv = astat.tile([128, 2], f32, tag="mv")
                nc.vector.bn_aggr(out=mv, in_=stats)
                std_t = astat.tile([128, 1], f32, tag="std")
                nc.scalar.activation(
                    out=std_t, in_=mv[:, 1:2],
                    func=mybir.ActivationFunctionType.Sqrt,
                    bias=eps_t, scale=1.0,
                )
                rstd_t = astat.tile([128, 1], f32, tag="rstd")
                nc.vector.reciprocal(out=rstd_t, in_=std_t)
                # xn = rstd*x + (-mean*rstd), applied on ScalarE
                nmr_t = astat.tile([128, 1], f32, tag="nmr")
                nc.vector.scalar_tensor_tensor(
                    out=nmr_t, in0=mv[:, 0:1], scalar=-1.0, in1=rstd_t,
                    op0=mybir.AluOpType.mult, op1=mybir.AluOpType.mult,
                )
                xn_t = atmp.tile([128, D], bf16, tag="xn")
                nc.scalar.activation(
                    out=xn_t, in_=x_t,
                    func=mybir.ActivationFunctionType.Identity,
                    scale=rstd_t, bias=nmr_t,
                )
                # transpose to xnT on the DMA xbar (no PE involvement)
                nc.sync.dma_start_transpose(
                    out=xnT[:, :, i * 128 : (i + 1) * 128], in_=xn_t
                )
                # V projection for this block
                psv = pscm.tile([128, 512], f32, tag="st", bufs=2,
                                name=f"psv_{i}")
                for c in range(NCH):
                    nc.tensor.matmul(
                        psv,
                        xnT[:, c, i * 128 : (i + 1) * 128],
                        wv_t[:, c, :],
                        start=(c == 0),
                        stop=False,
                    )
                # += ones[s] x bv  (rank-1 bias update)
                nc.tensor.matmul(psv, vones_t, bv1_t, start=False, stop=True)
                nc.vector.tensor_copy(
                    vpp[:, i, :, 0:HD],
                    psv.rearrange("p (h v) -> p h v", v=HD),
                )
                # stage weight DMAs by first-need order, after this block's
                # own DMAs so they don't delay the critical path
                if i == 0:
                    ensure_wqk(0, 0), ensure_wqk(1, 0)
                elif i == 4:
                    ensure_wqk(0, 1), ensure_wqk(1, 1)
                elif i == 6:
                    for c in range(4):
                        nc.sync.dma_start(out=wout_t[:, c, :],
                                          in_=wout_d[c, :, :])
                elif i == 8:
                    ensure_wqk(0, 2), ensure_wqk(1, 2)
                elif i == 10:
                    ensure_wqk(0, 3), ensure_wqk(1, 3)

            # ================= QKV q/k projection chunks =================
            wqk_tiles = {}

            def ensure_wqk(t, p):
                if (t, p) not in wqk_tiles:
                    fb = t * 4 + p
                    w_t = wqkp.tile([128, NCH, 128], bf16, tag="wqk",
                                    name=f"wqk_{t}_{p}")
                    nc.sync.dma_start(
                        out=w_t,
                        in_=wqk_d[:, :, fb * 128 : (fb + 1) * 128].rearrange(
                            "c d f -> d c f"
                        ),
                    )
                    wqk_tiles[(t, p)] = w_t
                return wqk_tiles[(t, p)]

            qk_done = set()

            def emit_qk_chunk(t, p, sb):
                if (t, p, sb) in qk_done:
                    return False
                qk_done.add((t, p, sb))
                w_t = ensure_wqk(t, p)
                dest = qT if t == 0 else kT
                ps = pscm.tile([128, 512], f32, tag="st", bufs=2,
                               name=f"psqk_{t}_{p}_{sb}")
                for c in range(NCH):
                    nc.tensor.matmul(
                        ps,
                        w_t[:, c, :],
                        xnT[:, c, sb * 512 : (sb + 1) * 512],
                        start=(c == 0),
                        stop=(c == NCH - 1),
                    )
                nc.vector.tensor_scalar_add(
                    out=dest[:, p, sb * 512 : (sb + 1) * 512],
                    in0=ps,
                    scalar1=bqk_t[:, t, p : p + 1],
                )
                return True

            # ============== deferred-work drip queue (PE filler) ==============
            pending = []        # (cost_ns, ready_fn, fn); fn->False = no-op
            budget = [0.0]
            seg_emitted = [0]   # LN segments fully emitted so far

            def drip(ns):
                budget[0] += ns
                while True:
                    pick = None
                    for idx, (c, rdy, fn) in enumerate(pending):
                        if c <= budget[0] and rdy():
                            pick = idx
                            break
                    if pick is None:
                        break
                    c, _, fn = pending.pop(pick)
                    if fn() is not False:
                        budget[0] -= c
                budget[0] = min(budget[0], 2500.0)

            def flush_pending():
                while pending:
                    _, _, fn = pending.pop(0)
                    fn()

            def qk_item(t, p, sb):
                return (1700.0, lambda: seg_emitted[0] > sb,
                        lambda: emit_qk_chunk(t, p, sb))

            def emit_outproj(i):
                y_t = ypool.tile([128, 1024], f32, tag="y", name=f"y_{i}")
                for nh in range(2):
                    pso = pscm.tile([128, 512], f32, tag="pso",
                                    bufs=2, name=f"pso_{i}_{nh}")
                    for c in range(4):
                        nc.tensor.matmul(
                            pso,
                            ytall[:, c, i * 128 : (i + 1) * 128],
                            wout_t[:, c, nh * 512 : (nh + 1) * 512],
                            start=(c == 0),
                            stop=(c == 3),
                        )
                    nc.vector.tensor_copy(
                        y_t[:, nh * 512 : (nh + 1) * 512], pso
                    )
                nc.sync.dma_start(
                    out=out_d[i * 128 : (i + 1) * 128, :], in_=y_t
                )

            # ===================== causal attention =====================
            pvq = []            # software-pipeline queue of PV closures

            def att_pair(sb, p):
                """Attention for (superblock sb, head pair p): scores + exp
                per k-block j; PV matmuls lag one j via pvq; epilogue
                (reciprocal + broadcast normalize + xbar transposes to ytall)
                rides the same queue."""
                q0 = sb * 512
                jmax = 4 * sb + 3
                yacc = pscm.tile([128, 1024], f32, tag="pv", bufs=1,
                                 name=f"yacc_{sb}_{p}")
                for j in range(jmax + 1):
                    r = max(0, j - 4 * sb)
                    diag = j >= 4 * sb
                    L = 512 - 128 * r
                    # hf1 always in the second PSUM bank: a start=True clears
                    # accumulation state bank-wide, so concurrent groups must
                    # not share a bank
                    hb1 = 512
                    st = pscm.tile([128, 1024], f32, tag="st", bufs=2,
                                   name=f"st_{p}_{sb}_{j}")
                    pt = ptp.tile([128, 1024], bf16, tag="pt")
                    for hf in range(2):
                        hrows = slice(hf * HD, (hf + 1) * HD)
                        lo = hf * hb1
                        nc.tensor.matmul(
                            st[:, lo : lo + L],
                            kT[hrows, p, j * 128 : (j + 1) * 128],
                            qT[hrows, p, q0 + r * 128 : q0 + 512],
                            start=True, stop=not diag,
                        )
                    if diag:
                        # causal mask folded in on the PE: st += I.T @ maskT
                        for hf in range(2):
                            lo = hf * hb1
                            nc.tensor.matmul(
                                st[:, lo : lo + 128],
                                identb, maskTb,
                                start=False, stop=True,
                            )
                    # one wide exp across both heads (for r=1 the [L:512)
                    # strip is unread garbage)
                    W = hb1 + L
                    nc.scalar.activation(
                        out=pt[:, 0:W], in_=st[:, 0:W],
                        func=mybir.ActivationFunctionType.Exp,
                    )

                    def emit_pv(j=j, r=r, hb1=hb1, pt=pt, yacc=yacc):
                        # j=0 covers all 4 q-blocks; only the first slot per
                        # bank may use start=True (it clears the whole bank's
                        # has_written bits; the other slots' first writes then
                        # overwrite, subsequent ones accumulate)
                        for hf in range(2):
                            for qb in range(r, 4):
                                off = hf * 512 + qb * 65
                                nc.tensor.matmul(
                                    yacc[:, off : off + HD + 1],
                                    pt[:, hf * hb1 + (qb - r) * 128 :
                                       hf * hb1 + (qb - r + 1) * 128],
                                    vpp[:, j, 2 * p + hf, :],
                                    start=(j == 0 and qb == 0),
                                    stop=(j == 4 * sb + qb),
                                    skip_group_check=True,
                                )

                    pvq.append(emit_pv)
                    while len(pvq) > 2:
                        pvq.pop(0)()
                    drip(1200.0 if j <= 1 else 400.0)

                def emit_epilogue(yacc=yacc, sb=sb, p=p, q0=q0):
                    yap = yacc[:, :]
                    pstride = list(yap.ap[0])
                    sums_ap = bass.AP(
                        tensor=yap.tensor, offset=yap.offset + HD,
                        ap=[pstride, [512, 2], [65, 4]],
                    )
                    rec = ctmp.tile([128, 2, 4], f32, tag="rec")
                    nc.vector.reciprocal(out=rec, in_=sums_ap)
                    # normalized y packed [q, qb, hf*64+hd] so each q-block is
                    # a contiguous [128,128] xbar-transposable tile
                    y_norm = ctmp.tile([128, 4, 128], bf16, tag="ynorm")
                    ynap = y_norm[:, :, :]
                    y_out_ap = bass.AP(
                        tensor=ynap.tensor, offset=ynap.offset,
                        ap=[list(ynap.ap[0]), [64, 2], [128, 4], [1, HD]],
                    )
                    y_in_ap = bass.AP(
                        tensor=yap.tensor, offset=yap.offset,
                        ap=[pstride, [512, 2], [65, 4], [1, HD]],
                    )
                    rap = rec[:, :, :]
                    rec_bcast = bass.AP(
                        tensor=rap.tensor, offset=rap.offset,
                        ap=[list(rap.ap[0]), [4, 2], [1, 4], [0, HD]],
                    )
                    nc.vector.tensor_tensor(
                        out=y_out_ap, in0=y_in_ap, in1=rec_bcast,
                        op=mybir.AluOpType.mult,
                    )
                    for qb in range(4):
                        nc.sync.dma_start_transpose(
                            out=ytall[:, p, q0 + qb * 128 : q0 + (qb + 1) * 128],
                            in_=y_norm[:, qb, :],
                        )
                    if p == 3:
                        for ib in range(4 * sb, 4 * sb + 4):
                            pending.append(
                                (1900.0, lambda: True,
                                 lambda ib=ib: emit_outproj(ib))
                            )

                # runs after the last PV of this pair drains out of pvq
                pvq.append(emit_epilogue)

            # ========================= schedule =========================
            # LN/V segments interleaved with pair-0 QKV pieces + attention;
            # pairs 1-3 QKV chunks drip in as filler during attention.
            for p in range(1, 4):
                for sb in range(NQS):
                    pending.append(qk_item(0, p, sb))
                    pending.append(qk_item(1, p, sb))

            for seg in range(4):
                # pair-0 q/k psums held in the (still unused) pso slots;
                # each 128-column piece depends only on one LN block
                qk_done.add((0, 0, seg))
                qk_done.add((1, 0, seg))
                ps_seg = [
                    pscm.tile([128, 512], f32, tag="pso", bufs=2,
                              name=f"psqk_{t}_0_{seg}")
                    for t in range(2)
                ]
                for b, i in enumerate(range(4 * seg, 4 * seg + 4)):
                    emit_A_block(i)
                    for t in range(2):
                        w_t = wqk_tiles[(t, 0)]
                        for c in range(NCH):
                            nc.tensor.matmul(
                                ps_seg[t][:, b * 128 : (b + 1) * 128],
                                w_t[:, c, :],
                                xnT[:, c, i * 128 : (i + 1) * 128],
                                start=(c == 0),
                                stop=(c == NCH - 1),
                            )
                    drip(500.0)
                for t in range(2):
                    nc.vector.tensor_scalar_add(
                        out=(qT if t == 0 else kT)[
                            :, 0, seg * 512 : (seg + 1) * 512
                        ],
                        in0=ps_seg[t],
                        scalar1=bqk_t[:, t, 0:1],
                    )
                seg_emitted[0] += 1
                att_pair(seg, 0)
            for p in range(1, 4):
                for sb in range(NQS):
                    for sb2 in range(sb + 1):
                        emit_qk_chunk(0, p, sb2)
                        emit_qk_chunk(1, p, sb2)
                    att_pair(sb, p)
            while pvq:
                pvq.pop(0)()
            flush_pending()

    nc.finalize()
    return nc


def _prep_core_inputs(x, ln_scale, ln_bias, w_qkv, b_qkv, w_out):
    """Host-side folding + per-core input maps."""
    scale = np.float32(HD ** -0.5)
    # qkv = xn@W + b_qkv, xn = z*ln_scale + ln_bias  =>  z @ (ln_scale*W) + (ln_bias@W + b_qkv)
    b_eff = b_qkv + np.einsum(
        "d,dhf->hf", ln_bias.astype(np.float64), w_qkv.astype(np.float64)
    ).astype(np.float32)
    w_eff = ln_scale[:, None, None] * w_qkv
    wq = w_eff[:, :, 0:64] * scale
    wk = w_eff[:, :, 64:128]
    wv = w_eff[:, :, 128:192]
    bq = b_eff[:, 0:64] * scale
    bk = b_eff[:, 64:128]
    bv = b_eff[:, 128:192]

    in_maps = []
    for core in range(8):
        b, g = core // 2, core % 2
        hsel = slice(g * HL, (g + 1) * HL)
        # [D, 4 pairs, 128] with head 2p in rows 0:64, head 2p+1 in 64:128
        qp = wq[:, hsel].reshape(D, 4, 128)
        kp = wk[:, hsel].reshape(D, 4, 128)
        wqk = np.concatenate(
            [qp.reshape(D, 512), kp.reshape(D, 512)], axis=1
        ).reshape(NCH, 128, 1024)
        wv_g = np.ascontiguousarray(wv[:, hsel].reshape(D, 512)).reshape(
            NCH, 128, 512
        )
        bq_p = bq[hsel].reshape(4, 128)
        bk_p = bk[hsel].reshape(4, 128)
        bqk = np.ascontiguousarray(
            np.stack([bq_p, bk_p], axis=0).transpose(2, 0, 1)
        )
        bv1 = np.ascontiguousarray(bv[hsel].reshape(1, 512))
        wout = np.ascontiguousarray(
            w_out[g * 512 : (g + 1) * 512, :].reshape(4, 128, 1024)
        )
        in_maps.append(
            {
                "x": np.ascontiguousarray(x[b]),
                "wqk": np.ascontiguousarray(wqk).astype(bfloat16),
                "wv": wv_g.astype(bfloat16),
                "bqk": bqk,
                "bv1": bv1,
                "vones": np.ones((1, 128), np.float32),
                "wout": wout.astype(bfloat16),
            }
        )
    return in_maps


def kernel(x, mask, ln_scale, ln_bias, w_qkv, b_qkv, w_out, b_out, **run_kwargs):
    x = np.asarray(x, np.float32)
    ln_scale = np.asarray(ln_scale, np.float32)
    ln_bias = np.asarray(ln_bias, np.float32)
    w_qkv = np.asarray(w_qkv, np.float32)
    b_qkv = np.asarray(b_qkv, np.float32)
    w_out = np.asarray(w_out, np.float32)
    b_out = np.asarray(b_out, np.float32)
    if "nc" not in _cache:
        _cache["nc"] = build_program()
    nc = _cache["nc"]
    in_maps = _prep_core_inputs(x, ln_scale, ln_bias, w_qkv, b_qkv, w_out)
    res = run_bass_kernel_spmd(nc, in_maps, list(range(8)), **run_kwargs)
    _cache["last_result"] = res
    out = np.empty((B, S, D), np.float32)
    for b in range(B):
        out[b] = res.results[2 * b]["out"] + res.results[2 * b + 1]["out"]
    out += np.asarray(b_out)[None, None, :]
    return out
